# revision 10
# baseline (speedup 1.0000x reference)
# Triplane FCDecoder kernel for 8x TRN2 NeuronCores.
#
# Math: out[b,n] = sum_{pl} bilinear(plane_pl[b], uv_pl(p[b,n])) . fc_w[:128]
#                  + p[b,n,:] . fc_w[128:131] + fc_b
# The decoder is linear, so each plane is projected through fc_w[:128]
# first ([1,128]x[128,HW] matmul), turning 100 MB of plane features into
# twelve 128x128 scalar tables T.  Bilinear sampling then needs the 4
# corner values T[s], T[s+1], T[s+128], T[s+129] per query point.
#
# Gather design: ap_gather broadcasts each of a Q7 core's indices to all
# 16 of its SBUF partitions (out[p,i] = tab[p][idx_core[i]]).  We load
# the 16 rows of core c with the 4 corner-SHIFTED copies x 4 batches of
# the plane's table (row 16c+4j+b = T[pl,b][d_j:], d = [0,1,128,129]).
# One d=1 bf16 index per (point, plane) then fetches all 4 corners at
# once (in 4 partitions).  Bilinear weights are applied in that spread
# layout and a constant [128]->[32] PE matmul sums the 4 corner rows.
#
# Point layout per core: Q7 core c, batch b, sub-row j' in [0,4),
# slot m in [0, M=392): compact partition P = 16c+4b+j', stream slot
# i = 16m + 4b + j' (so the idx tile IS the compact layout).  The
# gathered corner j of that point lands at [16c+4j+b, i].
#
# Sharding: query points split 8 ways (12544/batch/core padded); the
# projection reads each core's 1/8 column shard of all 12 tables; an
# AllGather replicates the projected tables.  Host sums the 3 per-plane
# partial results and adds the tiny p . fc_w[128:131] + fc_b term.

import numpy as np

B, N, C, RES = 4, 100000, 128, 128
NCORES = 8
HW = RES * RES
NP = N // NCORES            # points per batch per core (12500)
M = 392                     # slots per (core,batch,j') row
NPB = 32 * M                # padded points per batch per core (12544)
NI = 16 * M                 # ap_gather stream slots per Q7 core (6272)
COLS = HW // NCORES         # table column shard per core (2048)
PAD = 0.1
EPS = 1e-5
DELTA = (0, 1, RES, RES + 1)

_C1 = float(np.float32(RES - 1) / np.float32(1.0 + PAD + EPS))
_C2 = float(np.float32(0.5) * np.float32(RES - 1))
_XMAX = float(np.float32(np.float32(1.0 - EPS) * np.float32(RES - 1)))

_PLANES = [(0, 2), (0, 1), (1, 2)]  # xz, xy, yz

_prog_cache = {}

# timing knob: replicate the gather instruction per plane (slope method)
EXTRA_GATHER_REPS = 0

CHUNK = NI // 8             # spread-math column chunk (784)
PECH = 512                  # PE reduce chunk (PSUM bank)


def _build_program():
    import concourse.bacc as bacc
    import concourse.tile as tile
    import concourse.mybir as mybir
    import concourse.bass as cbass
    from concourse.bass import _add_dep_helper

    f32 = mybir.dt.float32
    f32r = mybir.dt.float32r
    bf16 = mybir.dt.bfloat16
    i32 = mybir.dt.int32
    i16 = mybir.dt.int16

    nc = bacc.Bacc(
        "TRN2",
        target_bir_lowering=False,
        debug=False,
        enable_asserts=False,
        num_devices=NCORES,
    )

    p_sw = nc.dram_tensor("p_sw", [128, M * 3], f32, kind="ExternalInput")
    sc = nc.dram_tensor("sc", [6, 128, NI], f32, kind="ExternalInput")
    pl_shard = nc.dram_tensor("pl_shard", [12, 128, COLS], f32, kind="ExternalInput")
    w_pl = nc.dram_tensor("w_pl", [128, 1], f32, kind="ExternalInput")
    bsel = nc.dram_tensor("bsel", [128, 32], f32, kind="ExternalInput")
    wconst = nc.dram_tensor("wconst", [128, 4], f32, kind="ExternalInput")
    out_d = nc.dram_tensor("out_sw", [3, 32, NI], f32, kind="ExternalOutput")

    with tile.TileContext(nc) as tc:
        with (
            tc.tile_pool(name="const", bufs=1) as constp,
            tc.tile_pool(name="work", bufs=1) as wk,
            tc.tile_pool(name="cb", bufs=1) as cbp,
            tc.tile_pool(name="stg", bufs=1) as stgp,
            tc.tile_pool(name="psum", bufs=1, space="PSUM") as psum,
            tc.tile_pool(name="dram", bufs=1, space="DRAM") as dram,
        ):
            # ---------------- phase 1: projection ----------------
            w_tile = constp.tile([128, 1], f32r)
            nc.sync.dma_start(w_tile[:], w_pl.ap().bitcast(f32r))
            bsel_t = constp.tile([128, 32], f32)
            nc.sync.dma_start(bsel_t[:], bsel.ap())
            wc_t = constp.tile([128, 4], f32)
            nc.sync.dma_start(wc_t[:], wconst.ap())

            shard_raw_d = dram.tile([12, COLS], f32)
            with tc.tile_pool(name="ph1", bufs=2) as ph1:
                for j in range(12):
                    stage = ph1.tile([1, COLS], f32, tag="stage")
                    for k in range(COLS // 512):
                        chunk = ph1.tile([128, 512], f32r, tag="chunk")
                        src = pl_shard.ap()[j, :, 512 * k: 512 * (k + 1)]
                        nc.sync.dma_start(chunk[:], src.bitcast(f32r))
                        pt = psum.tile([1, 512], f32, tag="pt")
                        nc.tensor.matmul(
                            pt[:], lhsT=w_tile[:], rhs=chunk[:], start=True, stop=True
                        )
                        nc.scalar.copy(stage[0:1, 512 * k: 512 * (k + 1)], pt[:])
                    nc.scalar.dma_start(shard_raw_d[j: j + 1], stage[:])

            # ---------------- phase 2: allgather + linearize ----------------
            ag_out = dram.tile([NCORES, 12, COLS], f32)
            nc.gpsimd.collective_compute(
                "AllGather",
                mybir.AluOpType.bypass,
                replica_groups=[list(range(NCORES))],
                ins=[shard_raw_d.opt()],
                outs=[ag_out.opt()],
            )
            # T_lin[j12, t] = ag_out[t // COLS, j12, t % COLS]
            t_lin = dram.tile([12, HW], f32)
            ag_ap = ag_out[:]
            src_lin = cbass.AP(
                tensor=ag_ap.tensor,
                offset=ag_ap.offset,
                ap=[[COLS, 12], [12 * COLS, NCORES], [1, COLS]],
            )
            lin_i = nc.sync.dma_start(t_lin[:], src_lin)

            # ---------------- phase 3: compact index math ----------------
            p_sb = constp.tile([128, M, 3], f32)
            nc.sync.dma_start(p_sb[:], p_sw.ap())

            idx_tiles = []
            with tc.tile_pool(name="idxwork", bufs=1) as iw:
                for pli, (ia, ib) in enumerate(_PLANES):
                    x0 = []
                    for coord in (ia, ib):
                        xt = iw.tile([128, M], f32, tag="xt")
                        nc.vector.tensor_scalar(
                            xt[:], p_sb[:, :, coord], _C1, _C2,
                            mybir.AluOpType.mult, mybir.AluOpType.add,
                        )
                        nc.vector.tensor_scalar(
                            xt[:], xt[:], 0.0, _XMAX,
                            mybir.AluOpType.max, mybir.AluOpType.min,
                        )
                        xi = iw.tile([128, M], i32, tag="xi")
                        nc.vector.tensor_copy(xi[:], xt[:])
                        xf = iw.tile([128, M], f32, tag=f"xf{coord}")
                        nc.vector.tensor_copy(xf[:], xi[:])
                        mk = iw.tile([128, M], f32, tag="mk")
                        nc.vector.tensor_tensor(
                            mk[:], xf[:], xt[:], mybir.AluOpType.is_gt)
                        nc.vector.tensor_tensor(
                            xf[:], xf[:], mk[:], mybir.AluOpType.subtract)
                        x0.append(xf)
                    st = iw.tile([128, M], f32, tag="st")
                    nc.vector.tensor_scalar(
                        st[:], x0[1][:], float(RES), None, mybir.AluOpType.mult)
                    nc.vector.tensor_tensor(
                        st[:], st[:], x0[0][:], mybir.AluOpType.add)
                    idx_t = constp.tile([128, M], i16, tag=f"idx{pli}")
                    nc.vector.tensor_copy(idx_t[:], st[:])
                    idx_tiles.append(idx_t)

            # ---------------- phase 4: per-plane pipeline ----------------
            tab_t = constp.tile([128, HW, 1], f32)
            g = constp.tile([128, NI, 1], f32)
            wsp = constp.tile([128, NI], f32)
            probe = constp.tile([128, 2], f32)
            mxp, sxp = wc_t[:, 0:1], wc_t[:, 1:2]
            myp, syp = wc_t[:, 2:3], wc_t[:, 3:4]

            tl_ap = t_lin[:]
            gathers = []

            def emit_dist(pli, tab):
                # tab[16c+4j+b] = T[pl,b][d_j:]; 32 contiguous 4-row DMAs
                dists = []
                for c8 in range(8):
                    for j, dj in enumerate(DELTA):
                        L = HW - dj
                        src = cbass.AP(
                            tensor=tl_ap.tensor,
                            offset=tl_ap.offset + (4 * pli) * HW + dj,
                            ap=[[HW, 4], [1, L]],
                        )
                        eng = nc.sync if ((c8 * 4 + j) % 2 == 0) else nc.scalar
                        di = eng.dma_start(
                            tab[16 * c8 + 4 * j: 16 * c8 + 4 * j + 4, 0:L, 0],
                            src)
                        _add_dep_helper(di.ins, lin_i.ins, True, "dist waits t_lin")
                        if gathers:
                            _add_dep_helper(di.ins, gathers[-1].ins, True,
                                            "tab rewrite waits prev gather")
                        dists.append(di)
                # probe: read the last sampled column of every row; the
                # gather depends on it so table data has fully landed
                pr = nc.vector.tensor_copy(probe[:, 0:1], tab[:, HW - 130:HW - 129, 0])
                for di in dists:
                    _add_dep_helper(pr.ins, di.ins, True, "probe waits dist")
                return dists, pr

            prev_readers = []
            for pli, (ia, ib) in enumerate(_PLANES):
                tab = tab_t
                dist_is, pr = emit_dist(pli, tab)

                # spread weights (chunked), concurrent with gather
                for ch in range(NI // CHUNK):
                    c0, c1 = ch * CHUNK, (ch + 1) * CHUNK
                    cu = cbp.tile([128, CHUNK], f32, tag=f"cu{ch % 2}")
                    cv = cbp.tile([128, CHUNK], f32, tag=f"cv{ch % 2}")
                    nc.sync.dma_start(cu[:], sc.ap()[2 * pli, :, c0:c1])
                    nc.sync.dma_start(cv[:], sc.ap()[2 * pli + 1, :, c0:c1])
                    fr = []
                    for src_t in (cu, cv):
                        a = wk.tile([128, CHUNK], f32, tag=f"wa{len(fr)}")
                        nc.vector.tensor_scalar(
                            a[:], src_t[:], _C1, _C2,
                            mybir.AluOpType.mult, mybir.AluOpType.add)
                        nc.vector.tensor_scalar(
                            a[:], a[:], 0.0, _XMAX,
                            mybir.AluOpType.max, mybir.AluOpType.min)
                        bi = wk.tile([128, CHUNK], i32, tag="wbi")
                        nc.vector.tensor_copy(bi[:], a[:])
                        cf = wk.tile([128, CHUNK], f32, tag="wcf")
                        nc.vector.tensor_copy(cf[:], bi[:])
                        dm = wk.tile([128, CHUNK], f32, tag="wdm")
                        nc.vector.tensor_tensor(
                            dm[:], cf[:], a[:], mybir.AluOpType.is_gt)
                        nc.vector.tensor_tensor(
                            cf[:], cf[:], dm[:], mybir.AluOpType.subtract)
                        nc.vector.tensor_tensor(
                            a[:], a[:], cf[:], mybir.AluOpType.subtract)
                        fr.append(a)
                    # t1 = fx*sx + mx (ACT, per-partition scale/bias)
                    nc.scalar.activation(
                        cu[:], fr[0][:], mybir.ActivationFunctionType.Identity,
                        bias=mxp, scale=sxp)
                    nc.scalar.activation(
                        cv[:], fr[1][:], mybir.ActivationFunctionType.Identity,
                        bias=myp, scale=syp)
                    nc.vector.tensor_tensor(
                        wsp[:, c0:c1], cu[:], cv[:], mybir.AluOpType.mult)

                # gather: one index per point fetches all 4 corners
                gi = nc.gpsimd.ap_gather(
                    g[:], tab[:], idx_tiles[pli][:],
                    channels=128, num_elems=HW, d=1, num_idxs=NI,
                )
                for di in dist_is:
                    _add_dep_helper(gi.ins, di.ins, True, "gather waits tables")
                _add_dep_helper(gi.ins, pr.ins, True, "gather waits probe")
                for rd in prev_readers:
                    _add_dep_helper(gi.ins, rd.ins, True, "gather waits g readers")
                for _rep in range(EXTRA_GATHER_REPS):
                    gx = nc.gpsimd.ap_gather(
                        g[:], tab[:], idx_tiles[pli][:],
                        channels=128, num_elems=HW, d=1, num_idxs=NI,
                    )
                    _add_dep_helper(gx.ins, pr.ins, True, "rep waits probe")
                    gi = gx
                gathers.append(gi)

                # combine: g *= wsp; PE reduces 4 corner rows -> [32, NI]
                g2d = g[:, :, 0]
                mu = nc.vector.tensor_tensor(
                    g2d, g2d, wsp[:], mybir.AluOpType.mult)
                _add_dep_helper(mu.ins, gi.ins, True, "mul waits gather")
                prev_readers = []
                nch = (NI + PECH - 1) // PECH
                for ch in range(nch):
                    c0 = ch * PECH
                    c1 = min(c0 + PECH, NI)
                    cw = c1 - c0
                    ps = psum.tile([32, cw], f32, tag=f"ps{ch % 4}")
                    mm = nc.tensor.matmul(
                        ps[:], lhsT=bsel_t[:], rhs=g2d[:, c0:c1],
                        start=True, stop=True,
                    )
                    prev_readers.append(mm)
                    stg = stgp.tile([32, PECH], f32, tag=f"st{ch % 2}")
                    nc.vector.tensor_copy(stg[:, 0:cw], ps[:])
                    eng = nc.sync if (ch % 2 == 0) else nc.scalar
                    eng.dma_start(out_d.ap()[pli, :, c0:c1], stg[:, 0:cw])

    nc.compile()
    return nc


def _get_program():
    if "nc" not in _prog_cache:
        _prog_cache["nc"] = _build_program()
    return _prog_cache["nc"]


def _pack_inputs(p, planes12, fc_w):
    # point (c, b, j', m): t = c*1568 + j'*392 + m, global n = r*NP + t
    # compact partition P = 16c + 4b + j'; stream slot i = 16m + 4b + j'
    # tab/spread rows: 16c + 4j + b (j = corner)
    in_maps = []
    w_pl_np = np.ascontiguousarray(fc_w[:128].reshape(128, 1))

    # bsel[p, g]: row p = 16c + 4j + b contributes to out row g = 4c + b
    pp = np.arange(128)
    bsel_np = np.zeros((128, 32), np.float32)
    bsel_np[pp, 4 * (pp // 16) + pp % 4] = 1.0

    # weight constants per row p: corner j = (p % 16) // 4
    wconst_np = np.zeros((128, 4), np.float32)
    jj = (pp % 16) // 4
    wconst_np[:, 0] = np.where(jj % 2 == 0, 1.0, 0.0)   # mx
    wconst_np[:, 1] = np.where(jj % 2 == 0, -1.0, 1.0)  # sx
    wconst_np[:, 2] = np.where(jj < 2, 1.0, 0.0)        # my
    wconst_np[:, 3] = np.where(jj < 2, -1.0, 1.0)       # sy

    for r in range(NCORES):
        p_r = np.zeros((B, NPB, 3), np.float32)
        p_r[:, :NP] = p[:, r * NP:(r + 1) * NP, :]
        # A[c, b, j', m, 3]
        A = p_r.reshape(B, 8, 4, M, 3).transpose(1, 0, 2, 3, 4)
        # compact [16c+4b+j', m, 3]
        p_compact = np.ascontiguousarray(A.reshape(128, M * 3))
        # spread coords per plane: us[16c+4j+b, 16m+4b+j'] = coord, all j
        sc_np = np.zeros((6, 128, NI), np.float32)
        for pli, (ia, ib) in enumerate(_PLANES):
            for ci, coord in enumerate((ia, ib)):
                # rows [c, j, b]; cols [m, b', j']
                u6 = np.zeros((8, 4, 4, M, 4, 4), np.float32)
                for b in range(B):
                    # A[c, b, j', m] -> [c, j(bcast), m, j']
                    u6[:, :, b, :, b, :] = A[:, b, :, :, coord].transpose(
                        0, 2, 1)[:, None, :, :]
                sc_np[2 * pli + ci] = u6.reshape(128, NI)
        in_maps.append({
            "p_sw": p_compact,
            "sc": sc_np,
            "pl_shard": np.ascontiguousarray(
                planes12[:, :, r * COLS:(r + 1) * COLS]),
            "w_pl": w_pl_np,
            "bsel": bsel_np,
            "wconst": wconst_np,
        })
    return in_maps


def kernel(p, c_xz, c_xy, c_yz, fc_w, fc_b, trace=False):
    from concourse import bass_utils

    nc = _get_program()

    p = np.asarray(p, dtype=np.float32)
    fc_w = np.asarray(fc_w, dtype=np.float32)
    fc_b = np.asarray(fc_b, dtype=np.float32)

    planes12 = np.empty((12, 128, HW), dtype=np.float32)
    for pli, c in enumerate([c_xz, c_xy, c_yz]):
        c = np.asarray(c, dtype=np.float32)
        planes12[pli * 4: pli * 4 + 4] = c.reshape(B, C, HW)

    in_maps = _pack_inputs(p, planes12, fc_w)

    res = bass_utils.run_bass_kernel_spmd(
        nc, in_maps, core_ids=list(range(NCORES)), trace=trace
    )
    if trace:
        print("exec_time_ns:", res.exec_time_ns)
        kernel.last_results = res

    out = np.empty((B, N), dtype=np.float32)
    for r in range(NCORES):
        o = res.results[r]["out_sw"].reshape(3, 32, NI).sum(axis=0)
        # o[4c+b, 16m+4b+j'] -> point (c, b, j', m)
        vv = o.reshape(8, 4, M, 4, 4)  # [c, b, m, b', j']
        for b in range(B):
            res_b = vv[:, b, :, b, :].transpose(0, 2, 1)  # [c, j', m]
            out[b, r * NP:(r + 1) * NP] = res_b.reshape(NPB)[:NP]
    out += p @ fc_w[128:131, 0] + fc_b[0]
    return out
